# revision 24
# baseline (speedup 1.0000x reference)
"""Trainium2 Bass kernel: single attention head (B=8, S=2048, E=1024, H=64).

Sharding: data-parallel over batch -- each of the 8 NeuronCores computes one
batch element's full attention. No collectives; every HBM byte read once.

v7: decoupled streaming pipeline (scheduling rules learned from traces):
  - Inputs staged host-side as fp16 quarter-slabs [NB, 128, EC*BW]; flat 2D
    DMA patterns -> cheap HWDGE triggers.  Weights packed into one slab,
    loaded first on the sync ring.  xk quarters get stream priority (every
    exp instruction transitively needs all of kt).
  - PE p-state warm-up: dummy matmuls on a memset tile before data lands.
  - ScalarE exp is the pole (~32 pair-instructions + 4 transpose-DMA
    triggers, ~1.24us each).  The schedule keeps every block's PE load at
    <= 48 matmuls (~6 per score pair) and never enqueues a matmul whose
    data cannot be present by its turn, so the exp stream never starves.
  - Scores per (block, key-tile-pair) into PSUM pair-tiles [128,2,512];
    AV pairs (ones-augmented v gives softmax denominators for free) are
    placed per vaug-quarter availability: tiles 0-7 of blocks 0/1 inside
    block 2, tiles 8-15 inside block 3, blocks 2/3's own AV in the tail
    chasing the exp stream.
  - Finalize per block: PSUM evac (vector) -> transpose-DMA on the
    (drained) sync ring -> reciprocal/scale (vector) -> store (gpsimd).
  - PSUM: scores 2x2 + kproj 2 (block-0 era) / oa 2 + qv-proj 1 = 8.
"""

import numpy as np

import concourse.bass as bass  # noqa: F401  (engine namespaces live on nc)
import concourse.mybir as mybir
import concourse.tile as tile
from concourse import bacc
from concourse.bass_utils import run_bass_kernel_spmd

B, S, E, H = 8, 2048, 1024, 64
EC = E // 128   # contraction chunks per projection
NT = S // 128   # key tiles
NB = 4          # 512-column blocks
BW = S // NB
F16 = mybir.dt.float16
F32 = mybir.dt.float32

_CACHE = {}


def _build_nc():
    nc = bacc.Bacc(None)
    xq = nc.declare_dram_parameter("xqs", [NB, 128, EC * BW], F16, isOutput=False)
    xk = nc.declare_dram_parameter("xks", [NB, 128, EC * BW], F16, isOutput=False)
    xv = nc.declare_dram_parameter("xvs", [NB, 128, EC * BW], F16, isOutput=False)
    ws = nc.declare_dram_parameter("ws", [128, 3 * EC * H], F16, isOutput=False)
    bq = nc.declare_dram_parameter("bq", [H, 1], F32, isOutput=False)
    bv = nc.declare_dram_parameter("bv", [H, 1], F32, isOutput=False)
    out = nc.declare_dram_parameter("out", [S, H], F32, isOutput=True)

    Exp = mybir.ActivationFunctionType.Exp

    with tile.TileContext(nc) as tc:
        with tc.tile_pool(name="const", bufs=1) as const, \
             tc.tile_pool(name="xqp", bufs=4) as xqp, \
             tc.tile_pool(name="xvp", bufs=4) as xvp, \
             tc.tile_pool(name="oassb", bufs=2) as oassb, \
             tc.tile_pool(name="trsp", bufs=2) as trsp, \
             tc.tile_pool(name="osbp", bufs=2) as osbp, \
             tc.tile_pool(name="rcp", bufs=2) as rcp:

            wsrc = const.tile([128, BW], F16, name="wsrc")
            nc.gpsimd.memset(wsrc[:], 0.0)
            bq_t = const.tile([H, 1], F32, name="bq_t")
            nc.gpsimd.dma_start(out=bq_t[:], in_=bq[:])
            bv_t = const.tile([H, 1], F32, name="bv_t")
            nc.gpsimd.dma_start(out=bv_t[:], in_=bv[:])

            kt = const.tile([64, S], F16, name="kt")
            qt = const.tile([64, S], F16, name="qt")
            vt = const.tile([64, S], F16, name="vt")
            vaug = const.tile([128, NT, 80], F16, name="vaug")
            warm = const.tile([1, 8], F16, name="warm")
            nc.gpsimd.memset(vaug[:, :, 64], 1.0)

            wsl = const.tile([128, 3, EC, H], F16, name="wsl")
            WIDX = {"k": 0, "q": 1, "v": 2}

            xq_b, xv_q = [], []
            for t in range(NB):
                xq_b.append(xqp.tile([128, EC, BW], F16, tag="xq", name=f"xq{t}"))
                xv_q.append(xvp.tile([128, EC, BW], F16, tag="xv", name=f"xv{t}"))

            def dma_slab(dst, dram, q):
                nc.sync.dma_start(
                    out=dst[:], in_=dram[q].rearrange("p (c s) -> p c s", c=EC))

            with tc.tile_pool(name="xkp", bufs=4) as xkp:
                xk_q = []
                for t in range(NB):
                    xk_q.append(
                        xkp.tile([128, EC, BW], F16, tag="xk", name=f"xk{t}"))

                nc.sync.dma_start(
                    out=wsl[:], in_=ws[:].rearrange("p (w c h) -> p w c h",
                                                    w=3, c=EC))
                dma_slab(xk_q[0], xk, 0)
                dma_slab(xq_b[0], xq, 0)
                dma_slab(xk_q[1], xk, 1)
                dma_slab(xk_q[2], xk, 2)
                dma_slab(xk_q[3], xk, 3)
                dma_slab(xq_b[1], xq, 1)
                dma_slab(xv_q[0], xv, 0)
                dma_slab(xq_b[2], xq, 2)
                dma_slab(xv_q[1], xv, 1)
                dma_slab(xq_b[3], xq, 3)
                dma_slab(xv_q[2], xv, 2)
                dma_slab(xv_q[3], xv, 3)

                # warm the Exp activation table off the critical path
                nc.scalar.activation(warm[:], wsrc[0:1, 0:8], Exp, scale=0.125)

                with tc.tile_pool(name="sps", bufs=2, space="PSUM") as sps, \
                     tc.tile_pool(name="pps", bufs=1, space="PSUM") as pps:

                    pts = [None] * NB
                    oas = [None] * NB

                    def wv_(nm, c):
                        return wsl[:, WIDX[nm], c, :]

                    def proj(w, xtile, dsti):
                        pp = pps.tile([64, BW], F32, tag="pp",
                                      name=f"pp{w}{dsti}")
                        for c in range(EC):
                            nc.tensor.matmul(
                                pp[:], wv_(w, c), xtile[:, c, :],
                                start=(c == 0), stop=(c == EC - 1),
                                skip_group_check=True)
                        bias = bq_t if w == "q" else bv_t
                        tgt = qt if w == "q" else vt
                        nc.vector.tensor_scalar_add(
                            tgt[:, dsti * BW:(dsti + 1) * BW], pp[:], bias[:])

                    def qproj(j):
                        proj("q", xq_b[j], j)

                    def vproj(q):
                        proj("v", xv_q[q], q)
                        # v^T quarter via transpose-DMA on the scalar ring;
                        # the trigger sits between exps at this emission spot
                        nc.scalar.dma_start_transpose(
                            vaug[:, 4 * q:4 * (q + 1), 0:64],
                            vt[:, q * BW:(q + 1) * BW])

                    def av_pair(j, t2):
                        for t in (t2, t2 + 1):
                            nc.tensor.matmul(
                                oas[j][:], vaug[:, t, 0:65], pts[j][:, t, :],
                                start=(t == 0), stop=(t == NT - 1),
                                skip_group_check=True)

                    def scores_pair(j, i):
                        st = sps.tile(
                            [128, 2, BW], F32, tag="st", name=f"st{j}_{i}")
                        for u in range(2):
                            nc.tensor.matmul(
                                st[:, u, :],
                                kt[:, (2 * i + u) * 128:(2 * i + u + 1) * 128],
                                qt[:, j * BW:(j + 1) * BW],
                                start=True, stop=True)
                        nc.scalar.activation(
                            pts[j][:, 2 * i:2 * i + 2, :], st[:],
                            Exp, scale=0.125)

                    def fin(j):
                        oasb = oassb.tile(
                            [80, BW], F16, tag="oasb", name=f"oasb{j}")
                        nc.vector.tensor_copy(oasb[0:65, :], oas[j][:])
                        trs = trsp.tile(
                            [128, 4, 80], F16, tag="trs", name=f"trs{j}")
                        nc.sync.dma_start_transpose(trs[:], oasb[:])
                        osb = osbp.tile(
                            [128, 4, H], F32, tag="osb", name=f"osb{j}")
                        for jj in range(4):
                            rc = rcp.tile(
                                [128, 1], F32, tag="rc", name=f"rc{j}_{jj}")
                            nc.vector.reciprocal(rc[:], trs[:, jj, 64:65])
                            nc.vector.tensor_scalar(
                                osb[:, jj, :], trs[:, jj, 0:64], rc[:], None,
                                op0=mybir.AluOpType.mult)
                        out_r = out[:].rearrange("(t p) h -> p t h", p=128)
                        nc.gpsimd.dma_start(
                            out=out_r[:, 4 * j:4 * (j + 1), :], in_=osb[:])

                    # ---- block 0 era: kproj pool (2 banks) open ----
                    with tc.tile_pool(name="kqp", bufs=2, space="PSUM") as kqp:

                        def kproj_q(t):
                            pk = kqp.tile(
                                [64, BW], F32, tag="kq", name=f"kq{t}")
                            for c in range(EC):
                                nc.tensor.matmul(
                                    pk[:], wv_("k", c), xk_q[t][:, c, :],
                                    start=(c == 0), stop=(c == EC - 1),
                                    skip_group_check=True)
                            nc.vector.tensor_copy(
                                kt[:, t * BW:(t + 1) * BW], pk[:])

                        dmy = kqp.tile([64, BW], F32, tag="kq", name="dmy")
                        for r in range(20):
                            nc.tensor.matmul(
                                dmy[:], wsrc[:, 0:64], wsrc[:, :],
                                start=True, stop=True, skip_group_check=True)

                        kproj_q(0)
                        qproj(0)
                        pts[0] = const.tile([128, NT, BW], F16, name="pt0")
                        for i in range(NT // 2):
                            scores_pair(0, i)
                            if i == 1:
                                kproj_q(1)
                            if i == 3:
                                kproj_q(2)
                            if i == 5:
                                kproj_q(3)

                    # ---- blocks 1-3: oa pool (2 banks) ----
                    with tc.tile_pool(name="oap", bufs=2, space="PSUM") as oap:
                        # block 1
                        qproj(1)
                        pts[1] = const.tile([128, NT, BW], F16, name="pt1")
                        for i in range(NT // 2):
                            scores_pair(1, i)
                            if i == 5:
                                vproj(0)

                        # block 2: AV tiles 0-7 of blocks 0/1
                        qproj(2)
                        pts[2] = const.tile([128, NT, BW], F16, name="pt2")
                        oas[0] = oap.tile([65, BW], F32, tag="oa", name="oa0")
                        oas[1] = oap.tile([65, BW], F32, tag="oa", name="oa1")
                        for i in range(NT // 2):
                            scores_pair(2, i)
                            if i == 0:
                                vproj(1)
                                av_pair(0, 0)
                            if i == 1:
                                av_pair(0, 2)
                            if i == 2:
                                av_pair(1, 0)
                            if i == 3:
                                av_pair(0, 4)
                            if i == 4:
                                av_pair(1, 2)
                            if i == 5:
                                av_pair(0, 6)
                            if i == 6:
                                vproj(2)
                                av_pair(1, 4)
                            if i == 7:
                                av_pair(1, 6)

                        # block 3: AV tiles 8-15 of blocks 0/1
                        qproj(3)
                        pts[3] = const.tile([128, NT, BW], F16, name="pt3")
                        for i in range(NT // 2):
                            scores_pair(3, i)
                            if i == 0:
                                vproj(3)
                            if i == 2:
                                av_pair(0, 8)
                            if i == 3:
                                av_pair(1, 8)
                            if i == 4:
                                av_pair(0, 10)
                                av_pair(1, 10)
                            if i == 5:
                                av_pair(0, 12)
                            if i == 6:
                                av_pair(0, 14)
                                av_pair(1, 12)
                            if i == 7:
                                av_pair(1, 14)

                        # tail: fins as each oa completes; av(2)/av(3) chase
                        fin(0)
                        oas[2] = oap.tile([65, BW], F32, tag="oa", name="oa2")
                        av_pair(2, 0)
                        av_pair(2, 2)
                        fin(1)
                        for p in range(2, 8):
                            av_pair(2, 2 * p)
                        oas[3] = oap.tile([65, BW], F32, tag="oa", name="oa3")
                        for p in range(8):
                            av_pair(3, 2 * p)
                        fin(2)
                        fin(3)

    nc.finalize()
    return nc


def get_nc():
    if "nc" not in _CACHE:
        _CACHE["nc"] = _build_nc()
    return _CACHE["nc"]


def _slab(x):
    # [S, E] f32 -> [NB, 128, EC*BW] f16, slab[q, p, c*BW+s] = x[q*BW+s, c*128+p]
    a = x.reshape(NB, BW, EC, 128).transpose(0, 3, 2, 1).astype(np.float16)
    return np.ascontiguousarray(a.reshape(NB, 128, EC * BW))


def _wslab(wk, wq, wv):
    # [E, H] x3 -> [128, 3*EC*H] f16, ws[p, w*EC*H + c*H + h] = W_w[c*128+p, h]
    stack = np.stack([wk, wq, wv], axis=0)          # [3, E, H]
    a = stack.reshape(3, EC, 128, H).transpose(2, 0, 1, 3)  # [128, 3, EC, H]
    return np.ascontiguousarray(a.reshape(128, 3 * EC * H).astype(np.float16))


def make_in_maps(inputs):
    q = np.asarray(inputs["query"], np.float32)
    k = np.asarray(inputs["key_"], np.float32)
    v = np.asarray(inputs["value"], np.float32)
    ws = _wslab(np.asarray(inputs["Wk"], np.float32),
                np.asarray(inputs["Wq"], np.float32),
                np.asarray(inputs["Wv"], np.float32))
    bq = np.ascontiguousarray(np.asarray(inputs["bq"], np.float32).reshape(H, 1))
    bv = np.ascontiguousarray(np.asarray(inputs["bv"], np.float32).reshape(H, 1))
    in_maps = []
    for b in range(B):
        in_maps.append({
            "xqs": _slab(q[b]),
            "xks": _slab(k[b]),
            "xvs": _slab(v[b]),
            "ws": ws,
            "bq": bq, "bv": bv,
        })
    return in_maps


def kernel(**inputs):
    nc = get_nc()
    in_maps = make_in_maps(inputs)
    res = run_bass_kernel_spmd(nc, in_maps, list(range(B)))
    return np.stack([res.results[b]["out"] for b in range(B)], axis=0)


# revision 28
# speedup vs baseline: 1.1471x; 1.1471x over previous
"""Trainium2 Bass kernel: single attention head (B=8, S=2048, E=1024, H=64).

Sharding: data-parallel over batch -- each of the 8 NeuronCores computes one
batch element's full attention. No collectives; every HBM byte read once.

v8: decoupled streaming pipeline (scheduling rules learned from traces):
  - Inputs staged host-side as fp16 quarter-slabs [NB, 128, EC*BW]; flat 2D
    DMA patterns -> cheap HWDGE triggers.  Weights packed into one slab,
    loaded first on the sync ring.
  - PE p-state warm-up via dummy matmuls on a memset tile; k projection
    quarters interleaved into block 0 (all-up-front would head-of-line
    block the first scores behind xk3's DMA).
  - ScalarE exp is the pole: 32 pair-instructions, ~1.15us each, and
    NOTHING else rides the scalar queue.  v^T tiles are built with PE
    transposes + vector copies (cheaper than transpose-DMA triggers that
    would stretch the exp stream).  Blocks stay <= 48 PE-matmuls and never
    enqueue work whose data cannot be present by its turn.
  - Scores per (block, key-tile-pair) into PSUM pair-tiles [128,2,512];
    AV pairs (ones-augmented v gives softmax denominators for free):
    av(0) inside block 2, av(1) + half of av(2) inside block 3, rest in a
    short tail chasing the exp stream; per-block finalize via PE transpose
    + reciprocal normalize, emitted as soon as each oa completes.
  - PSUM: scores 2x2 + qv-proj 1 + oa 2 + transpose 1 = 8 exactly
    (kproj uses 2 banks during block 0, before the oa pool opens).
"""

import numpy as np

import concourse.bass as bass  # noqa: F401  (engine namespaces live on nc)
import concourse.mybir as mybir
import concourse.tile as tile
from concourse import bacc
from concourse.bass_utils import run_bass_kernel_spmd
from concourse.masks import make_identity

B, S, E, H = 8, 2048, 1024, 64
EC = E // 128   # contraction chunks per projection
NT = S // 128   # key tiles
NB = 4          # 512-column blocks
BW = S // NB
F16 = mybir.dt.float16
F32 = mybir.dt.float32

_CACHE = {}


def _build_nc():
    nc = bacc.Bacc(None)
    xq = nc.declare_dram_parameter("xqs", [NB, 128, EC * BW], F16, isOutput=False)
    xk = nc.declare_dram_parameter("xks", [NB, 128, EC * BW], F16, isOutput=False)
    xv = nc.declare_dram_parameter("xvs", [NB, 128, EC * BW], F16, isOutput=False)
    ws = nc.declare_dram_parameter("ws", [128, 3 * EC * H], F16, isOutput=False)
    bq = nc.declare_dram_parameter("bq", [H, 1], F32, isOutput=False)
    bv = nc.declare_dram_parameter("bv", [H, 1], F32, isOutput=False)
    out = nc.declare_dram_parameter("out", [S, H], F32, isOutput=True)

    Exp = mybir.ActivationFunctionType.Exp

    with tile.TileContext(nc) as tc:
        with tc.tile_pool(name="const", bufs=1) as const, \
             tc.tile_pool(name="xqp", bufs=4) as xqp, \
             tc.tile_pool(name="xvp", bufs=4) as xvp, \
             tc.tile_pool(name="oassb", bufs=2) as oassb, \
             tc.tile_pool(name="osbp", bufs=2) as osbp, \
             tc.tile_pool(name="rcp", bufs=2) as rcp:

            wsrc = const.tile([128, BW], F16, name="wsrc")
            nc.gpsimd.memset(wsrc[:], 0.0)
            bq_t = const.tile([H, 1], F32, name="bq_t")
            nc.gpsimd.dma_start(out=bq_t[:], in_=bq[:])
            bv_t = const.tile([H, 1], F32, name="bv_t")
            nc.gpsimd.dma_start(out=bv_t[:], in_=bv[:])

            kt = const.tile([64, S], F16, name="kt")
            qt = const.tile([64, S], F16, name="qt")
            vt = const.tile([64, S], F16, name="vt")
            vaug = const.tile([128, NT, 80], F16, name="vaug")
            ident = const.tile([128, 128], F16, name="ident")
            warm = const.tile([1, 8], F16, name="warm")
            nc.gpsimd.memset(vaug[:, :, 64], 1.0)
            make_identity(nc, ident[:])

            wsl = const.tile([128, 3, EC, H], F16, name="wsl")
            WIDX = {"k": 0, "q": 1, "v": 2}

            xq_b, xv_q = [], []
            for t in range(NB):
                xq_b.append(xqp.tile([128, EC, BW], F16, tag="xq", name=f"xq{t}"))
                xv_q.append(xvp.tile([128, EC, BW], F16, tag="xv", name=f"xv{t}"))

            def dma_slab(dst, dram, q):
                nc.sync.dma_start(
                    out=dst[:], in_=dram[q].rearrange("p (c s) -> p c s", c=EC))

            with tc.tile_pool(name="xkp", bufs=4) as xkp:
                xk_q = []
                for t in range(NB):
                    xk_q.append(
                        xkp.tile([128, EC, BW], F16, tag="xk", name=f"xk{t}"))

                nc.sync.dma_start(
                    out=wsl[:], in_=ws[:].rearrange("p (w c h) -> p w c h",
                                                    w=3, c=EC))
                dma_slab(xk_q[0], xk, 0)
                dma_slab(xq_b[0], xq, 0)
                dma_slab(xk_q[1], xk, 1)
                dma_slab(xk_q[2], xk, 2)
                dma_slab(xq_b[1], xq, 1)
                dma_slab(xk_q[3], xk, 3)
                dma_slab(xv_q[0], xv, 0)
                dma_slab(xq_b[2], xq, 2)
                dma_slab(xv_q[1], xv, 1)
                dma_slab(xq_b[3], xq, 3)
                dma_slab(xv_q[2], xv, 2)
                dma_slab(xv_q[3], xv, 3)

                # warm the Exp activation table off the critical path
                nc.scalar.activation(warm[:], wsrc[0:1, 0:8], Exp, scale=0.125)

                with tc.tile_pool(name="sps", bufs=2, space="PSUM") as sps, \
                     tc.tile_pool(name="pps", bufs=1, space="PSUM") as pps, \
                     tc.tile_pool(name="trp", bufs=1, space="PSUM") as trp:

                    pts = [None] * NB
                    oas = [None] * NB

                    def wv_(nm, c):
                        return wsl[:, WIDX[nm], c, :]

                    def proj(w, xtile, dsti):
                        pp = pps.tile([64, BW], F32, tag="pp",
                                      name=f"pp{w}{dsti}")
                        for c in range(EC):
                            nc.tensor.matmul(
                                pp[:], wv_(w, c), xtile[:, c, :],
                                start=(c == 0), stop=(c == EC - 1),
                                skip_group_check=True)
                        bias = bq_t if w == "q" else bv_t
                        tgt = qt if w == "q" else vt
                        nc.vector.tensor_scalar_add(
                            tgt[:, dsti * BW:(dsti + 1) * BW], pp[:], bias[:])

                    def qproj(j):
                        proj("q", xq_b[j], j)

                    def vproj(q):
                        proj("v", xv_q[q], q)

                    def vaug_tr(q, half):
                        # two PE transposes + vector copies build half a
                        # vaug quarter (v^T tiles); stays off the exp queue
                        for jj in (2 * half, 2 * half + 1):
                            tr = trp.tile([128, 65], F16, tag="tr",
                                          name=f"vtr{q}_{jj}")
                            nc.tensor.transpose(
                                tr[:, 0:64],
                                vt[:, q * BW + jj * 128:q * BW + (jj + 1) * 128],
                                ident[0:64, 0:64])
                            nc.vector.tensor_copy(
                                vaug[:, 4 * q + jj, 0:64], tr[:, 0:64])

                    def av_pair(j, t2):
                        for t in (t2, t2 + 1):
                            nc.tensor.matmul(
                                oas[j][:], vaug[:, t, 0:65], pts[j][:, t, :],
                                start=(t == 0), stop=(t == NT - 1),
                                skip_group_check=True)

                    def scores_pair(j, i):
                        st = sps.tile(
                            [128, 2, BW], F32, tag="st", name=f"st{j}_{i}")
                        for u in range(2):
                            nc.tensor.matmul(
                                st[:, u, :],
                                kt[:, (2 * i + u) * 128:(2 * i + u + 1) * 128],
                                qt[:, j * BW:(j + 1) * BW],
                                start=True, stop=True)
                        nc.scalar.activation(
                            pts[j][:, 2 * i:2 * i + 2, :], st[:],
                            Exp, scale=0.125)

                    def fin(j):
                        oasb = oassb.tile(
                            [65, BW], F16, tag="oasb", name=f"oasb{j}")
                        nc.vector.tensor_copy(oasb[:], oas[j][:])
                        osb = osbp.tile(
                            [128, 4, H], F32, tag="osb", name=f"osb{j}")
                        for jj in range(4):
                            tr = trp.tile([128, 65], F16, tag="tr",
                                          name=f"ftr{j}_{jj}")
                            nc.tensor.transpose(
                                tr[:], oasb[:, jj * 128:(jj + 1) * 128],
                                ident[0:65, 0:65])
                            rc = rcp.tile(
                                [128, 1], F32, tag="rc", name=f"rc{j}_{jj}")
                            nc.vector.reciprocal(rc[:], tr[:, 64:65])
                            nc.vector.tensor_scalar(
                                osb[:, jj, :], tr[:, 0:64], rc[:], None,
                                op0=mybir.AluOpType.mult)
                        out_r = out[:].rearrange("(t p) h -> p t h", p=128)
                        nc.gpsimd.dma_start(
                            out=out_r[:, 4 * j:4 * (j + 1), :], in_=osb[:])

                    # ---- block 0 era: kproj pool (2 banks) open ----
                    with tc.tile_pool(name="kqp", bufs=2, space="PSUM") as kqp:

                        def kproj_q(t):
                            pk = kqp.tile(
                                [64, BW], F32, tag="kq", name=f"kq{t}")
                            for c in range(EC):
                                nc.tensor.matmul(
                                    pk[:], wv_("k", c), xk_q[t][:, c, :],
                                    start=(c == 0), stop=(c == EC - 1),
                                    skip_group_check=True)
                            nc.vector.tensor_copy(
                                kt[:, t * BW:(t + 1) * BW], pk[:])

                        dmy = kqp.tile([64, BW], F32, tag="kq", name="dmy")
                        for r in range(20):
                            nc.tensor.matmul(
                                dmy[:], wsrc[:, 0:64], wsrc[:, :],
                                start=True, stop=True, skip_group_check=True)

                        kproj_q(0)
                        qproj(0)
                        pts[0] = const.tile([128, NT, BW], F16, name="pt0")
                        for i in range(NT // 2):
                            scores_pair(0, i)
                            if i == 1:
                                kproj_q(1)
                            if i == 3:
                                kproj_q(2)
                            if i == 5:
                                kproj_q(3)

                    # ---- blocks 1-3: oa pool (2 banks) ----
                    with tc.tile_pool(name="oap", bufs=2, space="PSUM") as oap:
                        # block 1
                        qproj(1)
                        pts[1] = const.tile([128, NT, BW], F16, name="pt1")
                        for i in range(NT // 2):
                            scores_pair(1, i)
                            if i == 4:
                                vproj(0)
                            if i in (5, 6):
                                vaug_tr(0, i - 5)

                        # block 2: av(0) tiles 0-11 (quarter 3 not here yet)
                        qproj(2)
                        pts[2] = const.tile([128, NT, BW], F16, name="pt2")
                        oas[0] = oap.tile([65, BW], F32, tag="oa", name="oa0")
                        oas[1] = oap.tile([65, BW], F32, tag="oa", name="oa1")
                        for i in range(NT // 2):
                            scores_pair(2, i)
                            if i == 0:
                                vproj(1)
                                av_pair(0, 0)
                            if i == 1:
                                vaug_tr(1, 0)
                                av_pair(0, 2)
                            if i == 2:
                                vaug_tr(1, 1)
                                av_pair(0, 4)
                            if i == 3:
                                av_pair(0, 6)
                            if i == 4:
                                vproj(2)
                            if i == 5:
                                vaug_tr(2, 0)
                                av_pair(0, 8)
                            if i == 6:
                                vaug_tr(2, 1)
                                av_pair(0, 10)
                            if i == 7:
                                av_pair(1, 0)

                        # block 3: rest of av(0), all av(1), start av(2)
                        qproj(3)
                        pts[3] = const.tile([128, NT, BW], F16, name="pt3")
                        for i in range(NT // 2):
                            scores_pair(3, i)
                            if i == 0:
                                vproj(3)
                                av_pair(1, 2)
                            if i == 1:
                                vaug_tr(3, 0)
                                av_pair(1, 4)
                            if i == 2:
                                vaug_tr(3, 1)
                                av_pair(0, 12)
                            if i == 3:
                                av_pair(0, 14)
                                av_pair(1, 6)
                            if i == 4:
                                fin(0)
                                av_pair(1, 8)
                            if i == 5:
                                oas[2] = oap.tile(
                                    [65, BW], F32, tag="oa", name="oa2")
                                av_pair(1, 10)
                                av_pair(2, 0)
                            if i == 6:
                                av_pair(1, 12)
                                av_pair(2, 2)
                            if i == 7:
                                av_pair(1, 14)
                                av_pair(2, 4)

                        # tail
                        fin(1)
                        for p in range(3, 8):
                            av_pair(2, 2 * p)
                        oas[3] = oap.tile([65, BW], F32, tag="oa", name="oa3")
                        for p in range(8):
                            av_pair(3, 2 * p)
                        fin(2)
                        fin(3)

    nc.finalize()
    return nc


def get_nc():
    if "nc" not in _CACHE:
        _CACHE["nc"] = _build_nc()
    return _CACHE["nc"]


def _slab(x):
    # [S, E] f32 -> [NB, 128, EC*BW] f16, slab[q, p, c*BW+s] = x[q*BW+s, c*128+p]
    a = x.reshape(NB, BW, EC, 128).transpose(0, 3, 2, 1).astype(np.float16)
    return np.ascontiguousarray(a.reshape(NB, 128, EC * BW))


def _wslab(wk, wq, wv):
    # [E, H] x3 -> [128, 3*EC*H] f16, ws[p, w*EC*H + c*H + h] = W_w[c*128+p, h]
    stack = np.stack([wk, wq, wv], axis=0)          # [3, E, H]
    a = stack.reshape(3, EC, 128, H).transpose(2, 0, 1, 3)  # [128, 3, EC, H]
    return np.ascontiguousarray(a.reshape(128, 3 * EC * H).astype(np.float16))


def make_in_maps(inputs):
    q = np.asarray(inputs["query"], np.float32)
    k = np.asarray(inputs["key_"], np.float32)
    v = np.asarray(inputs["value"], np.float32)
    ws = _wslab(np.asarray(inputs["Wk"], np.float32),
                np.asarray(inputs["Wq"], np.float32),
                np.asarray(inputs["Wv"], np.float32))
    bq = np.ascontiguousarray(np.asarray(inputs["bq"], np.float32).reshape(H, 1))
    bv = np.ascontiguousarray(np.asarray(inputs["bv"], np.float32).reshape(H, 1))
    in_maps = []
    for b in range(B):
        in_maps.append({
            "xqs": _slab(q[b]),
            "xks": _slab(k[b]),
            "xvs": _slab(v[b]),
            "ws": ws,
            "bq": bq, "bv": bv,
        })
    return in_maps


def kernel(**inputs):
    nc = get_nc()
    in_maps = make_in_maps(inputs)
    res = run_bass_kernel_spmd(nc, in_maps, list(range(B)))
    return np.stack([res.results[b]["out"] for b in range(B)], axis=0)
